# revision 66
# baseline (speedup 1.0000x reference)
"""Trainium2 Bass kernel for nn_CAGetBoard (neural CA step).

Takes FULL inputs, shards batch across 8 NeuronCores (pure data parallel),
runs a Bass/Tile kernel per core, gathers the FULL output.

Per-core pipeline (B/8 images each), all matmuls bf16, software-pipelined
over a global stream of 8*n_img blocks (32 rows each):

  step n: stacks(n+1) | mm2(n-1)+u/x-fold | conv(n) | finish(n-6) | hooks

  - conv1 (Sobel folded into a 16->128 3x3 conv) = 2 accumulating matmuls
    (K=48 left-tap + K=96 center/right) x N=512, 16 single-bank PSUM accs
    per block, over a 6-copy row/col-shifted stacked x (bf16 DRAM
    scratch, cast in chunks pipelined one image ahead); 258-stride rows
    with one-time zeroed pad columns give W-edge SAME padding.
  - relu+bias drains PSUM->SBUF bf16, rotated ACT/DVE (q%3).
  - mm2 (128->16, zero-padded to M=32) col-tiled x4 at positions
    0/32/64/96, chunk interleave chk=4i+g -> [128,512] PSUM -> tanh
    drain; channel-3 rows stream back to row layout for the alive mask
    (plus an early row-128 extract to shorten the post0 chain).
  - alive + update masks in row layout; 3x3 binary dilation via banded
    bf16 matmuls + horizontal adds.
  - finishing in the packed (i,d,c) channel layout via per-i broadcast
    DMAs (plain partition-slice APs only -- partition-factored rearrange
    views break dependency tracking): t2 = x + d*u is computed in
    mm2_phase (not alive-gated); finish multiplies by alive, clips
    channels 0..2 per i in place, and stores 4 full-channel DMAs; bf16
    output cast to f32 on host.
"""

import numpy as np

import concourse.bass as bass
import concourse.bacc as bacc
import concourse.tile as tile
import concourse.mybir as mybir
from concourse.bass_utils import run_bass_kernel_spmd

dt = mybir.dt
F32 = dt.float32
BF16 = dt.bfloat16
AF = mybir.ActivationFunctionType
OP = mybir.AluOpType

N_CORES = 8
C = 16
H = 256
W = 256
TR = 32                    # rows per compute block
WS = W + 2                 # padded row stride
N_BLK = H // TR
PX_IMG = H * W
PX_BLK = TR * W            # 8192
EPS = 0.5
ALIVE_T = 0.1


def _build_consts(w1, b1, w2, b2):
    w1 = np.asarray(w1, np.float32)
    w2 = np.asarray(w2, np.float32)
    sob = np.array([[-1., 0., 1.], [-2., 0., 2.], [-1., 0., 1.]], np.float32)
    W1x, W1gx, W1gy = w1[:, 0:16], w1[:, 16:32], w1[:, 32:48]
    k1f = (W1gx[:, :, None, None] * sob[None, None, :, :]
           + W1gy[:, :, None, None] * sob.T[None, None, :, :])
    k1f[:, :, 1, 1] += W1x
    lhs = np.transpose(k1f, (3, 2, 1, 0)).reshape(3, 48, 128)
    lhsA = lhs[0].copy()
    lhsB = np.concatenate([lhs[1], lhs[2]], axis=0)

    w2t32 = np.zeros((128, 32), np.float32)
    w2t32[:, 0:16] = w2.T

    b2dup = np.zeros((128, 1), np.float32)
    for i in range(4):
        b2dup[32 * i:32 * i + 16, 0] = b2

    bandB = np.zeros((128, 128), np.float32)
    for k in range(128):
        bandB[k, max(0, k - 1):k + 2] = 1.0
    bandClo = np.zeros((128, 128), np.float32)
    bandClo[0, 127] = 1.0
    bandChi = np.zeros((128, 128), np.float32)
    bandChi[127, 0] = 1.0
    clo1 = np.zeros((1, 128), np.float32)
    clo1[0, 127] = 1.0

    # pack everything into two tensors (2 DMAs at startup)
    cbf = np.zeros((128, 800), np.float32)
    cbf[0:48, 0:128] = lhsA
    cbf[0:96, 128:256] = lhsB
    cbf[:, 256:288] = w2t32
    cbf[:, 288:416] = bandB
    cbf[:, 416:544] = bandClo
    cbf[:, 544:672] = bandChi
    cbf[0:1, 672:800] = clo1
    cf32 = np.concatenate(
        [np.asarray(b1, np.float32).reshape(128, 1), b2dup], axis=1)
    return dict(cbf=cbf, cf32=cf32)


CONST_SPECS = dict(
    cbf=([128, 800], BF16), cf32=([128, 2], F32),
)

# (name, row range, col range) views into the packed bf16 const tile
_CBF_VIEWS = dict(
    lhsA=(48, 0, 128), lhsB=(96, 128, 256), w2t32=(128, 256, 288),
    bandB=(128, 288, 416), bandClo=(128, 416, 544), bandChi=(128, 544, 672),
    clo1=(1, 672, 800),
)


def build_program(n_img, reps=1):
    nc = bacc.Bacc("TRN2", target_bir_lowering=False)

    x_d = nc.dram_tensor("x", [n_img, C, H, W], F32, kind="ExternalInput")
    rand_d = nc.dram_tensor("rand", [n_img, H, W], F32, kind="ExternalInput")
    cst_d = {k: nc.dram_tensor(k, sh, d, kind="ExternalInput")
             for k, (sh, d) in CONST_SPECS.items()}
    out_d = nc.dram_tensor("out", [n_img, C, H, W], BF16,
                           kind="ExternalOutput")
    alive_d = nc.dram_tensor("alivescr", [n_img, PX_IMG], BF16,
                             kind="Internal")
    u_d = nc.dram_tensor("uscr", [n_img, PX_IMG], BF16, kind="Internal")
    xbf_d = nc.dram_tensor("xbfscr", [n_img, C, H, W], BF16, kind="Internal")

    xf = x_d.ap().rearrange("b c h w -> b c (h w)")
    outf = out_d.ap().rearrange("b c h w -> b c (h w)")
    randf = rand_d.ap().rearrange("b h w -> b (h w)")

    with tile.TileContext(nc) as tc:
        xbf = xbf_d.ap().rearrange("b c h w -> b c (h w)")
        _emit(nc, tc, n_img, xf, randf, cst_d, outf, alive_d.ap(),
              u_d.ap(), xbf, reps)
    nc.compile()
    return nc


def _emit(nc, tc, n_img, xf, randf, cst_d, outf, alivef, uf, xbf, reps=1):
    from contextlib import ExitStack
    ctx = ExitStack()

    def pool(name, bufs, **kw):
        return ctx.enter_context(tc.tile_pool(name=name, bufs=bufs, **kw))

    consts = pool("consts", 1)
    stackp = pool("stack", 1)
    hgrp_p = pool("hgrp", 2)
    dgrp_p = pool("dgrp", 3)
    fin_p = pool("fin", 2)
    fs_p = pool("fs", 2)
    row_p = pool("rows", 4)
    rowsm_p = pool("rowsm", 2)
    misc_p = pool("misc", 1)
    conv_ps = pool("convps", 5, space="PSUM")
    mask_ps = pool("maskps", 1, space="PSUM")
    mm2_ps = pool("mm2ps", 2, space="PSUM")

    cbf_t = consts.tile([128, 800], BF16, tag="cbf", name="cbf")
    nc.scalar.dma_start(cbf_t[:], cst_d["cbf"].ap())
    cf32_t = consts.tile([128, 2], F32, tag="cf32", name="cf32")
    nc.scalar.dma_start(cf32_t[:], cst_d["cf32"].ap())
    cst = {k: cbf_t[0:p, c0:c1] for k, (p, c0, c1) in _CBF_VIEWS.items()}
    cst["b1c"] = cf32_t[:, 0:1]
    cst["b2dup"] = cf32_t[:, 1:2]

    # 3 rotating stack tiles [96, TR, WS].
    # Partitions 16*di+c      (A group): x row r0+r+di-1 at cols 1..257
    # Partitions 48+16*di+c   (B group): x row r0+r+di-1 at cols 0..256
    # matmul A reads [0:48,  r, 0:W]   -> left tap x[j-1]
    # matmul B reads [0:96,  r, 1:W+1] -> center (A part) + right (B part)
    # One-time pad zeroing: A col 0, B cols 256..258.
    stacks = []
    for s in range(3):
        st = stackp.tile([96, TR * WS], BF16, tag=f"stack{s}",
                         name=f"stack{s}")
        st3 = st.rearrange("p (r j) -> p r j", j=WS)
        nc.vector.memset(st3[0:48, :, 0:1], 0.0)
        nc.vector.memset(st3[0:96, :, W:W + 2], 0.0)
        stacks.append(st3)

    sdil = []
    for s in range(4):
        t = misc_p.tile([128, WS], F32, tag=f"sdil{s}", name=f"sdil{s}")
        nc.vector.memset(t[:, 0:1], 0.0)
        nc.vector.memset(t[:, W + 1:W + 2], 0.0)
        sdil.append(t)

    def dilate_half(half, b_main, extra_lhs, extra_rhs, out_t, sgrp=0):
        """out = dilate3x3(binary) for one 128-row half.
        vertical: bandB.T @ b_main + extra_lhs.T @ extra_rhs, then horizontal
        adds on a 258-padded drain tile, then > 0.5."""
        vs = mask_ps.tile([128, W], F32, tag="mask", name="vs")
        nc.tensor.matmul(vs[:], cst["bandB"][:], b_main[:],
                         start=True, stop=(extra_lhs is None))
        if extra_lhs is not None:
            nc.tensor.matmul(vs[:], extra_lhs, extra_rhs,
                             start=False, stop=True)
        s = sdil[2 * sgrp + half]
        nc.scalar.activation(s[:, 1:W + 1], vs[:], AF.Copy)
        t = rowsm_p.tile([128, W], F32, tag="dil_t", name="dil_t")
        nc.vector.tensor_add(t[:], s[:, 0:W], s[:, 2:W + 2])
        nc.vector.tensor_add(t[:], t[:], s[:, 1:W + 1])
        nc.vector.tensor_single_scalar(out_t[:], t[:], 0.5, OP.is_gt)

    # image sequence (reps for benchmarking only)
    seq = [i for _ in range(reps) for i in range(n_img)]
    n_seq = len(seq)
    NG = n_seq * N_BLK

    def cast_chunk(s, j, n_chunks=4):
        """Cast chunk j/n_chunks of image seq[s] to bf16 scratch."""
        b = seq[s]
        lo = j * (PX_IMG // n_chunks)
        hi = lo + PX_IMG // n_chunks
        nc.gpsimd.dma_start(xbf[b, :, lo:hi], xf[b, :, lo:hi])

    # per-image row-layout state, keyed by sequence index
    rows = {}

    def rowpass(s):
        b = seq[s]
        st = dict(x3row=[], randrow=[], bpre=[], prealive=[], d3row=[])
        rows[s] = st
        for half in range(2):
            xt = row_p.tile([128, W], F32, tag="x3row", name="x3row")
            nc.sync.dma_start(
                xt[:], xf[b, 3, half * 128 * W:(half + 1) * 128 * W]
                .rearrange("(p w) -> p w", w=W))
            st["x3row"].append(xt)
            rt = row_p.tile([128, W], F32, tag="randrow", name="randrow")
            nc.sync.dma_start(
                rt[:], randf[b, half * 128 * W:(half + 1) * 128 * W]
                .rearrange("(p w) -> p w", w=W))
            st["randrow"].append(rt)
            bt = row_p.tile([128, W], BF16, tag="bpre", name="bpre")
            nc.vector.tensor_single_scalar(bt[:], xt[:], ALIVE_T, OP.is_gt)
            st["bpre"].append(bt)
            st["prealive"].append(row_p.tile([128, W], BF16, tag="prealive",
                                             name="prealive"))
            st["d3row"].append(row_p.tile([128, W], BF16, tag="d3row",
                                          name="d3row"))
            if half == 0:
                st["d3e"] = row_p.tile([1, W], BF16, tag="d3e", name="d3e")
            ut = row_p.tile([128, W], BF16, tag="urow", name="urow")
            nc.vector.tensor_single_scalar(ut[:], rt[:], EPS, OP.is_lt)
            nc.scalar.dma_start(
                uf[b, half * 128 * W:(half + 1) * 128 * W]
                .rearrange("(p w) -> p w", w=W), ut[:])
        dilate_half(0, st["bpre"][0], cst["bandClo"][:], st["bpre"][1][:],
                    st["prealive"][0])
        dilate_half(1, st["bpre"][1], cst["bandChi"][:], st["bpre"][0][:],
                    st["prealive"][1])

    hkeep = {}
    t2keep = {}
    postkeep = {}

    def stacks_issue(g):
        s, blk = divmod(g, N_BLK)
        b = seq[s]
        r0 = blk * TR
        st3 = stacks[g % 3]
        if blk == 0:
            nc.vector.memset(st3[0:64, 0:1, :], 0.0)
        if blk == N_BLK - 1:
            nc.vector.memset(st3[32:64, TR - 1:TR, :], 0.0)
            nc.vector.memset(st3[64:96, TR - 1:TR, :], 0.0)
        for di in range(3):
            rr_lo = max(0, 1 - di - r0)
            rr_hi = min(TR, H - r0 - di + 1)
            src = xbf[b, :, (r0 + rr_lo + di - 1) * W:
                      (r0 + rr_hi + di - 1) * W].rearrange(
                          "c (r w) -> c r w", w=W)
            nc.sync.dma_start(
                st3[16 * di:16 * di + 16, rr_lo:rr_hi, 1:W + 1], src)
            nc.sync.dma_start(
                st3[48 + 16 * di:64 + 16 * di, rr_lo:rr_hi, 0:W], src)

    def conv_phase(g):
        st3 = stacks[g % 3]
        hgt = hgrp_p.tile([128, 8192], BF16, tag="hgt", name="hgt")
        hkeep[g] = hgt
        for q in range(16):
            acc = conv_ps.tile([128, 512], F32, tag="conv", name="conv")
            nc.tensor.matmul(acc[:], cst["lhsA"][:],
                             st3[0:48, 2 * q:2 * q + 2, 0:W],
                             start=True, stop=False)
            nc.tensor.matmul(acc[:], cst["lhsB"][:],
                             st3[0:96, 2 * q:2 * q + 2, 1:W + 1],
                             start=False, stop=True)
            hsl = hgt[:, 512 * q:512 * (q + 1)]
            if q % 3 == 1:
                nc.vector.tensor_scalar(hsl, acc[:], cst["b1c"],
                                        0.0, op0=OP.add, op1=OP.max)
            else:
                nc.scalar.activation(hsl, acc[:], AF.Relu,
                                     bias=cst["b1c"])

    def mm2_phase(g):
        s, blk = divmod(g, N_BLK)
        b = seq[s]
        px0 = blk * PX_BLK
        hgt = hkeep.pop(g)
        # dgb [128, 2048]: partition (i,d,c), col (grp,n);
        # px = px0 + 2048 i + 512 grp + n; d=1 partitions hold zeros
        dgb = dgrp_p.tile([128, 2048], BF16, tag="d", name="d")
        for grp in range(4):
            mm = mm2_ps.tile([128, 512], F32, tag="mm2", name="mm2")
            for i in range(4):
                nc.tensor.matmul(
                    mm[32 * i:32 * i + 32, :],
                    cst["w2t32"][:],
                    hgt[:, 2048 * i + 512 * grp:2048 * i + 512 * (grp + 1)],
                    start=True, stop=True,
                    tile_position=(0, 32 * i))
            nc.scalar.activation(dgb[:, 512 * grp:512 * (grp + 1)], mm[:],
                                 AF.Tanh, bias=cst["b2dup"])
            if grp == 0 and blk == 4:
                # early extract of image row 128 for bp128 (shortens the
                # post0 dependency chain by ~3 tanh drains + full d3)
                d3e = rows[s]["d3e"]
                nc.gpsimd.dma_start(d3e[:], dgb[3:4, 0:256])
        half = blk // 4
        d3t = rows[s]["d3row"][half]
        for i in range(4):
            r = (blk % 4) * 32 + 8 * i
            nc.gpsimd.dma_start(d3t[r:r + 8, :],
                                dgb[32 * i + 3:32 * i + 4, :])
        # u/x folding is not alive-gated: do it here, 5 steps before finish
        xd = fin_p.tile([128, 2048], BF16, tag="xdup", name="xdup")
        for i in range(4):
            eng = nc.sync if i % 2 == 0 else nc.gpsimd
            eng.dma_start(
                xd[32 * i:32 * i + 32, :],
                xbf[b, :, px0 + 2048 * i:px0 + 2048 * (i + 1)]
                .unsqueeze(0).broadcast_to([2, C, 2048]))
        u16 = fin_p.tile([128, 2048], BF16, tag="u16", name="u16")
        for i in range(2):
            nc.gpsimd.dma_start(
                u16[64 * i:64 * i + 64, :],
                uf[b, px0 + 4096 * i:px0 + 4096 * (i + 1)]
                .rearrange("(j m) -> j m", m=2048)
                .unsqueeze(1).broadcast_to([2, 32, 2048]))
        t = fs_p.tile([128, 2048], BF16, tag="t", name="t", bufs=2)
        t2 = fs_p.tile([128, 2048], BF16, tag="t2", name="t2", bufs=9)
        nc.vector.tensor_mul(t[:], dgb[:], u16[:])
        nc.vector.tensor_add(t2[:], t[:], xd[:])
        for i in range(4):
            nc.vector.tensor_scalar(t2[32 * i:32 * i + 3, :],
                                    t2[32 * i:32 * i + 3, :], 1.0, 0.0,
                                    op0=OP.min, op1=OP.max)
        t2keep[g] = t2

    def post_binary(rows_ap_rand, rows_ap_x3, d3_ap, out_t):
        """out = (x3 + d3*(rand<eps)) > 0.1  on row-layout tiles."""
        m = rowsm_p.tile(list(out_t.shape), F32, tag="postm", name="postm")
        nc.vector.scalar_tensor_tensor(
            m[:], rows_ap_rand, EPS, d3_ap,
            op0=OP.is_lt, op1=OP.mult)
        nc.vector.tensor_add(m[:], m[:], rows_ap_x3)
        nc.vector.tensor_single_scalar(out_t[:], m[:], ALIVE_T, OP.is_gt)

    def alive_store(b, half, ar):
        nc.scalar.dma_start(
            alivef[b, half * 128 * W:(half + 1) * 128 * W]
            .rearrange("(p w) -> p w", w=W), ar[:])

    def post0(s):
        b = seq[s]
        st = rows[s]
        bpost0 = rowsm_p.tile([128, W], BF16, tag="bpost0", name="bpost0")
        post_binary(st["randrow"][0][:], st["x3row"][0][:],
                    st["d3row"][0][:], bpost0)
        bp128 = rowsm_p.tile([1, W], BF16, tag="bp128", name="bp128")
        post_binary(st["randrow"][1][0:1, :], st["x3row"][1][0:1, :],
                    st["d3e"][0:1, :], bp128)
        postal0 = rowsm_p.tile([128, W], BF16, tag="postal0", name="postal0")
        dilate_half(0, bpost0, cst["clo1"][:], bp128[:], postal0, sgrp=1)
        ar0 = rowsm_p.tile([128, W], BF16, tag="ar0", name="ar0")
        nc.vector.tensor_mul(ar0[:], st["prealive"][0][:], postal0[:])
        alive_store(b, 0, ar0)
        postkeep[s] = bpost0

    def post1(s):
        b = seq[s]
        st = rows[s]
        bpost0 = postkeep.pop(s)
        bpost1 = rowsm_p.tile([128, W], BF16, tag="bpost1", name="bpost1")
        post_binary(st["randrow"][1][:], st["x3row"][1][:],
                    st["d3row"][1][:], bpost1)
        postal1 = rowsm_p.tile([128, W], BF16, tag="postal1", name="postal1")
        dilate_half(1, bpost1, cst["bandChi"][:], bpost0[:], postal1, sgrp=1)
        ar1 = rowsm_p.tile([128, W], BF16, tag="ar1", name="ar1")
        nc.vector.tensor_mul(ar1[:], st["prealive"][1][:], postal1[:])
        alive_store(b, 1, ar1)

    def finish_block(g):
        s, blk = divmod(g, N_BLK)
        b = seq[s]
        px0 = blk * PX_BLK
        t2 = t2keep.pop(g)
        a16 = fin_p.tile([128, 2048], BF16, tag="a16", name="a16")
        for i in range(2):
            nc.scalar.dma_start(
                a16[64 * i:64 * i + 64, :],
                alivef[b, px0 + 4096 * i:px0 + 4096 * (i + 1)]
                .rearrange("(j m) -> j m", m=2048)
                .unsqueeze(1).broadcast_to([2, 32, 2048]))
        t3 = fs_p.tile([128, 2048], BF16, tag="t3", name="t3")
        nc.vector.tensor_mul(t3[:], t2[:], a16[:])
        for i in range(4):
            nc.sync.dma_start(
                outf[b, :, px0 + 2048 * i:px0 + 2048 * (i + 1)],
                t3[32 * i:32 * i + 16, :])

    # ---------------- pipeline ----------------
    cast_chunk(0, 0, 8)
    cast_chunk(0, 1, 8)
    stacks_issue(0)
    cast_chunk(0, 2, 8)
    rowpass(0)
    for step in range(NG + 9):
        if step + 1 < NG:
            stacks_issue(step + 1)
        if step >= 1 and step - 1 < NG:
            mm2_phase(step - 1)
        if step < NG:
            conv_phase(step)
        # hooks
        if step <= 4:
            cast_chunk(0, step + 3, 8)
        if step % N_BLK in (2, 3, 4, 5) and step // N_BLK + 1 < n_seq:
            cast_chunk(step // N_BLK + 1, step % N_BLK - 2)
        if (step + 2) % N_BLK == 0 and (step + 2) // N_BLK < n_seq:
            rowpass((step + 2) // N_BLK)
        if step >= 8 and (step - 8) % N_BLK == 0 \
                and (step - 8) // N_BLK < n_seq:
            post0((step - 8) // N_BLK)
        if step >= 11 and (step - 11) % N_BLK == 0 \
                and (step - 11) // N_BLK < n_seq:
            post1((step - 11) // N_BLK)
        if 0 <= step - 8 < NG:
            finish_block(step - 8)

    ctx.close()


# ---------------------------------------------------------------------------

_NC_CACHE = {}


def _get_nc(n_img, reps=1):
    key = (n_img, reps)
    if key not in _NC_CACHE:
        _NC_CACHE[key] = build_program(n_img, reps)
    return _NC_CACHE[key]


def kernel(x, w1, b1, w2, b2, rand_mask):
    x = np.ascontiguousarray(np.asarray(x, np.float32))
    rand_mask = np.ascontiguousarray(np.asarray(rand_mask, np.float32))
    B = x.shape[0]
    n_img = B // N_CORES
    consts = _build_consts(w1, b1, w2, b2)
    cast = {k: np.ascontiguousarray(v.astype(mybir.dt.np(CONST_SPECS[k][1])))
            for k, v in consts.items()}

    nc = _get_nc(n_img)
    in_maps = []
    for k in range(N_CORES):
        sl = slice(k * n_img, (k + 1) * n_img)
        in_maps.append(dict(x=x[sl], rand=rand_mask[sl, 0], **cast))
    res = run_bass_kernel_spmd(nc, in_maps, core_ids=list(range(N_CORES)))
    out = np.concatenate([res.results[k]["out"] for k in range(N_CORES)],
                         axis=0)
    return out.astype(np.float32)
